# revision 13
# baseline (speedup 1.0000x reference)
"""Single-head attention (B=8, S=2048, E=1024, H=128) with softmax + deterministic
dropout, data-parallel over batch across 8 NeuronCores (one batch element per core).

Per-core layout strategy ("transposed attention"):
  - host ships xT = x[b].T           [E, S]  fp16 (contraction dim E on partitions)
  - host ships keepT = keep[b].T     [S, S]  fp16 {0,1} (dropout mask, t-major)
  - qT/kT/vT[h, s] = w.T @ xT        (PE fp16, fp32 PSUM)
  - v natural [t, h] via 16 PE transposes of vT (fp16)
  - attT[t, s] = k[t-chunk] @ qT     (PE fp16; lhsT = kT chunk, rhs = qT slice)
  - expT = exp(attT * E^-0.5)        (ACT, PSUM -> fp16 SBUF)
  - denomT[1, s] += ones.T @ expT    (PE fp16, M=1, accumulated over t-chunks)
  - attd = expT * keepT              (DVE fp16, 2x mode)
  - outT[h, s] += v[t-chunk].T @ attd  (PE fp16, fp32 PSUM accumulation)
  - normalize by 1/(0.9*denom) fused into the final PSUM->SBUF copy (ACT
    per-partition scale) after PE-transposing outT back to natural [s, h].

Precision: fp16 rounding on x/w/q/k contributes only ~3e-5 to the softmax
logits (their absolute scale is ~0.1 after the E^-0.5 scaling); the fp16
value path (v, exp, attd) dominates at ~2-4e-4 L2 on the output, with all
contractions accumulated in fp32 PSUM.

The s dimension is processed in pairs of 512-wide groups (one fp32 PSUM bank
each): att/exp/dropout run 1024 wide to amortize instruction overheads, while
matmuls stay N=512 (PSUM bank limit) and out/denom accumulate per 512 group.
"""

import sys

for _p in ("/opt/trn_rl_repo",):
    if _p not in sys.path:
        sys.path.append(_p)

import numpy as np

B, S, E, H = 8, 2048, 1024, 128
DROP_P = 0.1
P = 128

_program_cache = {}


def _build_program(S=S, E=E):
    key = (S, E)
    if key in _program_cache:
        return _program_cache[key]
    NT = S // P   # t-chunks
    NE = E // P   # e-chunks
    SG = 512      # accumulation group width (one fp32 PSUM bank)
    PW = 1024     # processing width (sg pair)
    NPAIR = S // PW
    NC4 = SG // P

    import concourse.bass as bass  # noqa: F401
    import concourse.mybir as mybir
    import concourse.tile as tile
    from concourse import bacc
    from concourse.masks import make_identity

    f32 = mybir.dt.float32
    f16 = mybir.dt.float16
    Exp = mybir.ActivationFunctionType.Exp
    Copy = mybir.ActivationFunctionType.Copy
    SCALE = float(E) ** -0.5

    nc = bacc.Bacc("TRN2", target_bir_lowering=False, debug=False)
    xT_d = nc.dram_tensor("xT", [E, S], f16, kind="ExternalInput").ap()
    keepT_d = nc.dram_tensor("keepT", [S, S], f16, kind="ExternalInput").ap()
    wq_d = nc.dram_tensor("wq", [E, H], f16, kind="ExternalInput").ap()
    wk_d = nc.dram_tensor("wk", [E, H], f16, kind="ExternalInput").ap()
    wv_d = nc.dram_tensor("wv", [E, H], f16, kind="ExternalInput").ap()
    out_d = nc.dram_tensor("out", [S, H], f32, kind="ExternalOutput").ap()

    xT_r = xT_d.rearrange("(eo p) s -> p eo s", p=P)
    w_rs = [w.rearrange("(eo p) h -> p eo h", p=P) for w in (wq_d, wk_d, wv_d)]
    # keepT viewed as [p, t_chunk, s] so one DMA loads a whole s-column block
    keepT_r = keepT_d.rearrange("(to p) s -> p to s", p=P)

    with tile.TileContext(nc) as tc:
        with (
            tc.tile_pool(name="consts", bufs=1) as consts,
            tc.tile_pool(name="xw", bufs=1) as xw_pool,
            tc.tile_pool(name="qkv", bufs=1) as qkv_pool,
        ):
            identity = consts.tile([P, P], f32)
            make_identity(nc, identity)
            identity16 = consts.tile([P, P], f16)
            nc.any.tensor_copy(identity16, identity)
            ones_t = consts.tile([P, 1], f16)
            nc.vector.memset(ones_t, 1.0)

            # -------- load weights first (small, gates the first matmul),
            # then x^T as per-e tiles so projections start as chunks land ----
            w_sb = xw_pool.tile([P, 3, NE, H], f16)
            for j in range(3):
                nc.sync.dma_start(w_sb[:, j], w_rs[j])
            xT_es = []
            for e in range(NE):
                xe = xw_pool.tile([P, S], f16, tag=f"x{e}", name=f"x{e}")
                nc.sync.dma_start(xe, xT_r[:, e, :])
                xT_es.append(xe)

            # -------- projections: qT/kT [H, S] f16; vT -> v natural f16 ----
            qkT_sb = qkv_pool.tile([P, 2, S], f16)  # [h, (q|k), s]
            vT_sb = qkv_pool.tile([P, S], f16)
            v_sb = qkv_pool.tile([P, NT, H], f16)   # v natural: [t_in, t_chunk, h]
            with (
                tc.tile_pool(name="proj_ps", bufs=3, space="PSUM") as proj_ps,
                tc.tile_pool(name="tr_ps", bufs=2, space="PSUM") as tr_ps,
            ):
                for c in range(S // SG):
                    for j in range(3):
                        ps = proj_ps.tile([P, SG], f32, tag="proj")
                        for e in range(NE):
                            nc.tensor.matmul(
                                ps,
                                w_sb[:, j, e, :],
                                xT_es[e][:, c * SG:(c + 1) * SG],
                                start=(e == 0),
                                stop=(e == NE - 1),
                            )
                        if j < 2:
                            nc.any.tensor_copy(qkT_sb[:, j, c * SG:(c + 1) * SG], ps)
                        else:
                            nc.any.tensor_copy(vT_sb[:, c * SG:(c + 1) * SG], ps)
                # v natural via PE transpose of vT (fp16, exact for fp16 data)
                for t in range(NT):
                    ps_v = tr_ps.tile([P, P], f16, tag="vtr")
                    nc.tensor.transpose(
                        ps_v, vT_sb[:, t * P:(t + 1) * P], identity16
                    )
                    nc.any.tensor_copy(v_sb[:, t, :], ps_v)

            # -------- main attention loop over s-group pairs --------
            with (
                tc.tile_pool(name="att_ps", bufs=2, space="PSUM") as att_ps,
                tc.tile_pool(name="out_ps", bufs=1, space="PSUM") as out_ps,
                tc.tile_pool(name="den_ps", bufs=1, space="PSUM") as den_ps,
                tc.tile_pool(name="keep_pool", bufs=2) as keep_pool,
                tc.tile_pool(name="sb", bufs=3) as sb_pool,
                tc.tile_pool(name="sb2", bufs=2) as sb2_pool,
            ):
                for pr in range(NPAIR):
                    s_lo = pr * PW
                    p_sl = slice(s_lo, s_lo + PW)
                    # one big strided DMA: dropout mask for every t at this pair
                    keep_pr = keep_pool.tile([P, NT, PW], f16, tag="keep")
                    nc.sync.dma_start(keep_pr, keepT_r[:, :, p_sl])
                    psum_outs = [out_ps.tile([P, SG], f32, tag=f"out{h}",
                                             name=f"out{h}") for h in range(2)]
                    psum_dens = [den_ps.tile([1, SG], f32, tag=f"den{h}",
                                             name=f"den{h}") for h in range(2)]
                    expTs = {}
                    attds = {}

                    def emit_front(t, p_sl=p_sl, keep_pr=keep_pr,
                                   expTs=expTs, attds=attds):
                        psum_att = att_ps.tile([P, PW], f32, tag="att", name=f"att{t}")
                        for h in range(2):
                            nc.tensor.matmul(
                                psum_att[:, h * SG:(h + 1) * SG],
                                qkT_sb[:, 1, t * P:(t + 1) * P],  # kT chunk [H, 128]
                                qkT_sb[:, 0, s_lo + h * SG:s_lo + (h + 1) * SG],
                                start=True,
                                stop=True,
                            )
                        expT = sb_pool.tile([P, PW], f16, tag="exp", name=f"exp{t}")
                        nc.scalar.activation(expT, psum_att, Exp, scale=SCALE)
                        attd = sb_pool.tile([P, PW], f16, tag="attd", name=f"attd{t}")
                        nc.vector.tensor_mul(out=attd, in0=expT, in1=keep_pr[:, t, :])
                        expTs[t] = expT
                        attds[t] = attd

                    def emit_back(t, psum_outs=psum_outs, psum_dens=psum_dens,
                                  expTs=expTs, attds=attds):
                        expT = expTs.pop(t)
                        attd = attds.pop(t)
                        for h in range(2):
                            h_sl = slice(h * SG, (h + 1) * SG)
                            nc.tensor.matmul(
                                psum_dens[h],
                                ones_t,
                                expT[:, h_sl],
                                start=(t == 0),
                                stop=(t == NT - 1),
                            )
                            nc.tensor.matmul(
                                psum_outs[h],
                                v_sb[:, t, :],
                                attd[:, h_sl],
                                start=(t == 0),
                                stop=(t == NT - 1),
                            )

                    # software pipeline: back-stage ops run one iteration behind
                    # the att matmul so PE never waits on ACT/DVE results.
                    for t in range(NT):
                        emit_front(t)
                        if t >= 1:
                            emit_back(t - 1)
                    emit_back(NT - 1)

                    for h in range(2):
                        sg_lo = s_lo + h * SG
                        # denominator -> natural-layout 1/(0.9*den) [s_in, 1]
                        den_sb = sb2_pool.tile([1, SG], f32, tag="den_sb")
                        nc.scalar.mul(den_sb, psum_dens[h], 1.0 - DROP_P)
                        outT_sb = sb2_pool.tile([P, SG], f32, tag="outT")
                        nc.any.tensor_copy(outT_sb, psum_outs[h])
                        recip_nat = sb2_pool.tile([P, NC4], f32, tag="recip")
                        for c in range(NC4):
                            ps_rt = att_ps.tile([P, PW], f32, tag="att", name="ps_rt")
                            ps_r = ps_rt[:, 0:1]
                            nc.tensor.transpose(
                                ps_r, den_sb[:, c * P:(c + 1) * P],
                                identity[0:1, 0:1]
                            )
                            nc.vector.reciprocal(recip_nat[:, c:c + 1], ps_r)
                        # transpose outT to natural [s, h]; scale by recip
                        for c in range(NC4):
                            ps_ot = att_ps.tile([P, PW], f32, tag="att", name="ps_ot")
                            ps_o = ps_ot[:, 0:P]
                            nc.tensor.transpose(
                                ps_o, outT_sb[:, c * P:(c + 1) * P], identity
                            )
                            out_nat = sb2_pool.tile([P, H], f32, tag="out_nat")
                            nc.scalar.activation(
                                out_nat, ps_o, Copy, scale=recip_nat[:, c:c + 1]
                            )
                            row = sg_lo + c * P
                            nc.sync.dma_start(out_d[row:row + P, :], out_nat)

    nc.compile()
    _program_cache[key] = nc
    return nc


def kernel(x, wq, wk, wv, drop_u):
    from concourse import bass_utils

    x = np.asarray(x)
    wq = np.asarray(wq)
    wk = np.asarray(wk)
    wv = np.asarray(wv)
    drop_u = np.asarray(drop_u)

    nc = _build_program()
    in_maps = build_in_maps(x, wq, wk, wv, drop_u)
    res = bass_utils.run_bass_kernel_spmd(
        nc, in_maps, core_ids=list(range(B)), trace=False
    )
    return np.stack([res.results[b]["out"] for b in range(B)], axis=0)


def build_in_maps(x, wq, wk, wv, drop_u):
    wq16 = np.asarray(wq).astype(np.float16)
    wk16 = np.asarray(wk).astype(np.float16)
    wv16 = np.asarray(wv).astype(np.float16)
    in_maps = []
    for b in range(B):
        xT = np.ascontiguousarray(x[b].T).astype(np.float16)
        keepT = np.ascontiguousarray(
            (drop_u[b].T >= np.float32(DROP_P)).astype(np.float16)
        )
        in_maps.append(
            {"xT": xT, "keepT": keepT, "wq": wq16, "wk": wk16, "wv": wv16}
        )
    return in_maps
